# revision 22
# baseline (speedup 1.0000x reference)
"""Trainium2 Bass kernel for nn_Attention_10797547782838.

Windowed multi-head attention with per-query angle bias:
  q = (x@Wq+bq) reshaped to heads; k,v = x@Wkv+bkv
  attn = (q*scale) @ k^T * anglebias(q) + mask[b%4]; softmax; @v; proj Wp.

Sharding: batch (16) data-parallel over 8 cores, 2 batches/core.

Device design (v2):
- S^T layout ([k on partitions, q free]); P = exp(S^T) * exp(mask)^T.
- The angle bias is constant for this problem (angle_table == 1), so
  scale*bias is folded into Wq on the host (with a general fallback that
  multiplies a per-(c,q) bias tensor if it ever isn't constant).
- All matmul inputs fp16 (same mantissa as fp32r, 2x faster at 128-row
  stationaries) except the attention P/V path which stays bf16 for range.
- Softmax rowsums come FREE from the AV matmul: the V stationary is
  padded to [V_h | 1 | 0...] (64 wide), two heads per PSUM bank at column
  offsets 0/64, so each accumulated po bank holds O^T rows 0-31/64-95,
  rowsums at rows 32/96, and zeros elsewhere. No separate ones-matmul
  pass over P (which previously cost as much as the AV itself).
- Normalization: reciprocal_approx_fast on the po bank, then two tiny
  ones[1,64]-stationary matmuls broadcast 1/rowsum across partitions;
  one full-tile DVE multiply normalizes; junk partitions are exactly 0.
- Output projection uses host-permuted Wp with zero rows at the junk
  partitions, so the sparse otn layout multiplies straight through.
"""
import math
import sys

import numpy as np

sys.path.insert(0, "/opt/trn_rl_repo")

import concourse.bass as bass  # noqa: E402
import concourse.bacc as bacc  # noqa: E402
import concourse.tile as tile  # noqa: E402
from concourse import mybir  # noqa: E402
from concourse.bass_utils import run_bass_kernel_spmd  # noqa: E402

F32 = mybir.dt.float32
F32R = mybir.dt.float32r
F16 = mybir.dt.float16
BF16 = mybir.dt.bfloat16
AF = mybir.ActivationFunctionType
ALU = mybir.AluOpType

B, N, C = 16, 1024, 256
HEADS, HD = 8, 32
NW = 4
N_CORES = 8
BPC = B // N_CORES  # batches per core

_CACHE = {}


def r32(ap):
    return ap.bitcast(F32R)


def build_kernel(with_bias):
    from contextlib import ExitStack
    nc = bacc.Bacc("TRN2", target_bir_lowering=False, debug=False,
                   num_devices=N_CORES)

    d_xT = nc.dram_tensor("xT", [BPC, C, N], F16, kind="ExternalInput").ap()
    d_emT = nc.dram_tensor("emT", [BPC, N, N], BF16, kind="ExternalInput").ap()
    d_wq = nc.dram_tensor("wq", [C, C], F16, kind="ExternalInput").ap()
    d_wk = nc.dram_tensor("wk", [C, C], F16, kind="ExternalInput").ap()
    d_wv = nc.dram_tensor("wv", [C, C], F16, kind="ExternalInput").ap()
    d_wp = nc.dram_tensor("wp", [4, 128, C], BF16, kind="ExternalInput").ap()
    d_bq = nc.dram_tensor("bq", [128, 2], F32, kind="ExternalInput").ap()
    d_bk = nc.dram_tensor("bk", [128, 2], F32, kind="ExternalInput").ap()
    d_bv = nc.dram_tensor("bv", [128, C], F32, kind="ExternalInput").ap()
    d_bp = nc.dram_tensor("bp", [128, C], F32, kind="ExternalInput").ap()
    d_ones = nc.dram_tensor("ones64", [128, 2, 64], BF16, kind="ExternalInput").ap()
    d_biasT = None
    if with_bias:
        d_biasT = nc.dram_tensor("biasT", [BPC, C, N], F32,
                                 kind="ExternalInput").ap()
    d_y = nc.dram_tensor("y", [BPC, N, C], F32, kind="ExternalOutput").ap()

    with tile.TileContext(nc) as tc:
        with ExitStack() as ctx, nc.allow_low_precision(reason="fp16/bf16 matmul inputs; accumulation stays fp32 in PSUM"):
            kernel_body(ctx, tc, d_xT, d_emT, d_wq, d_wk, d_wv, d_wp,
                        d_bq, d_bk, d_bv, d_bp, d_ones, d_biasT, d_y)
    nc.compile()
    return nc


def kernel_body(ctx, tc, d_xT, d_emT, d_wq, d_wk, d_wv, d_wp,
                d_bq, d_bk, d_bv, d_bp, d_ones, d_biasT, d_y):
    nc = tc.nc

    consts = ctx.enter_context(tc.tile_pool(name="consts", bufs=1))
    vpool = ctx.enter_context(tc.tile_pool(name="vpool", bufs=1))
    xpool = ctx.enter_context(tc.tile_pool(name="xpool", bufs=2))
    empool = ctx.enter_context(tc.tile_pool(name="empool", bufs=2))
    qkpool = ctx.enter_context(tc.tile_pool(name="qkpool", bufs=2))
    ppool = ctx.enter_context(tc.tile_pool(name="ppool", bufs=4))
    otpool = ctx.enter_context(tc.tile_pool(name="otpool", bufs=2))
    rpool = ctx.enter_context(tc.tile_pool(name="rpool", bufs=2))
    ypool = ctx.enter_context(tc.tile_pool(name="ypool", bufs=3))
    psS = ctx.enter_context(tc.tile_pool(name="psS", bufs=2, space="PSUM"))
    psO = ctx.enter_context(tc.tile_pool(name="psO", bufs=4, space="PSUM"))

    # ---- constants / weights (once) ----
    w_sb = {}
    for name, dram in (("wq", d_wq), ("wk", d_wk), ("wv", d_wv)):
        t = consts.tile([128, 2, C], F16, tag=f"w_{name}")
        nc.sync.dma_start(out=t[:], in_=dram.rearrange("(c p) n -> p c n", p=128))
        w_sb[name] = t
    wp_sb = consts.tile([128, 4, C], BF16, tag="w_wp")
    nc.sync.dma_start(out=wp_sb[:], in_=d_wp.rearrange("t p n -> p t n"))
    bq_sb = consts.tile([128, 2], F32, tag="bq")
    nc.sync.dma_start(out=bq_sb[:], in_=d_bq[:])
    bk_sb = consts.tile([128, 2], F32, tag="bk")
    nc.sync.dma_start(out=bk_sb[:], in_=d_bk[:])
    bv_sb = consts.tile([128, C], F32, tag="bv")
    nc.sync.dma_start(out=bv_sb[:], in_=d_bv[:])
    bp_sb = consts.tile([128, C], F32, tag="bp")
    nc.sync.dma_start(out=bp_sb[:], in_=d_bp[:])
    ones_sb = consts.tile([128, 2, 64], BF16, tag="ones")
    nc.sync.dma_start(out=ones_sb[:], in_=d_ones[:])
    # v_aug[(parity, kc)]: [128 tokens, 8 heads, 64] bf16 = [V_h | 1 | 0...]
    v_aug = {}
    for par in range(2):
        for kc in range(8):
            t = vpool.tile([128, HEADS, 64], BF16, tag=f"vaug{par}_{kc}")
            nc.vector.memset(t[:, :, 32:64], 1.0)
            v_aug[(par, kc)] = t

    mm_ctr = [0]

    for b in range(BPC):
        par = b % 2
        # ---- stage inputs ----
        xT_sb = xpool.tile([128, 2, N], F16, tag="xT")
        nc.sync.dma_start(out=xT_sb[:], in_=d_xT[b].rearrange("(c p) n -> p c n", p=128))
        em_sb = empool.tile([128, 8, N], BF16, tag="em")
        nc.sync.dma_start(out=em_sb[:], in_=d_emT[b].rearrange("(kc p) q -> p kc q", p=128))
        if d_biasT is not None:
            biasT_sb = xpool.tile([128, 2, N], F32, tag="biasT")
            nc.sync.dma_start(out=biasT_sb[:],
                              in_=d_biasT[b].rearrange("(c p) n -> p c n", p=128))

        # ---- q/k projections -> fp16 [128, 2(m), N] ----
        qT_sb = qkpool.tile([128, 2, N], F16, tag="qT")
        kT_sb = qkpool.tile([128, 2, N], F16, tag="kT")
        for m in range(2):
            ps_q = psS.tile([128, N], F32, tag="s")
            for qc2 in range(2):
                for ci in range(2):
                    nc.tensor.matmul(
                        ps_q[:, qc2 * 512:(qc2 + 1) * 512],
                        w_sb["wq"][:, ci, m * 128:(m + 1) * 128],
                        xT_sb[:, ci, qc2 * 512:(qc2 + 1) * 512],
                        start=(ci == 0), stop=(ci == 1))
            if d_biasT is not None:
                nc.vector.scalar_tensor_tensor(
                    out=qT_sb[:, m, :], in0=ps_q[:], scalar=bq_sb[:, m:m + 1],
                    in1=biasT_sb[:, m, :], op0=ALU.add, op1=ALU.mult)
            else:
                nc.vector.tensor_scalar_add(
                    out=qT_sb[:, m, :], in0=ps_q[:], scalar1=bq_sb[:, m:m + 1])
            ps_k = psS.tile([128, N], F32, tag="s")
            for qc2 in range(2):
                for ci in range(2):
                    nc.tensor.matmul(
                        ps_k[:, qc2 * 512:(qc2 + 1) * 512],
                        w_sb["wk"][:, ci, m * 128:(m + 1) * 128],
                        xT_sb[:, ci, qc2 * 512:(qc2 + 1) * 512],
                        start=(ci == 0), stop=(ci == 1))
            nc.vector.tensor_scalar_add(
                out=kT_sb[:, m, :], in0=ps_k[:], scalar1=bk_sb[:, m:m + 1])

        # ---- V projection -> v_aug [128, h, 0:32] bf16 ----
        for g in range(2):
            ps_v = psS.tile([128, N], F32, tag="s")
            for k4 in range(4):
                kc = 4 * g + k4
                for ci in range(2):
                    nc.tensor.matmul(
                        ps_v[:, k4 * 256:(k4 + 1) * 256],
                        xT_sb[:, ci, kc * 128:(kc + 1) * 128],
                        w_sb["wv"][:, ci, :],
                        start=(ci == 0), stop=(ci == 1))
            for k4 in range(4):
                kc = 4 * g + k4
                nc.vector.tensor_tensor(
                    out=v_aug[(par, kc)][:, :, 0:32],
                    in0=ps_v[:, k4 * 256:(k4 + 1) * 256].rearrange(
                        "p (h c) -> p h c", h=HEADS),
                    in1=bv_sb[:].rearrange("p (h c) -> p h c", h=HEADS),
                    op=ALU.add)

        # ---- attention ----
        otn_sb = otpool.tile([128, 4, N], BF16, tag="otn")
        for qc in range(2):
            qsl = slice(qc * 512, (qc + 1) * 512)
            po = [psO.tile([128, 512], F32, tag="po", name=f"po{_t}")
                  for _t in range(4)]
            for kc in range(8):
                for t in range(4):
                    ps_s = psS.tile([128, N], F32, tag="s")
                    for hh in range(2):
                        h = 2 * t + hh
                        m, j = h // 4, h % 4
                        nc.tensor.matmul(
                            ps_s[:, hh * 512:(hh + 1) * 512],
                            kT_sb[32 * j:32 * (j + 1), m, kc * 128:(kc + 1) * 128],
                            qT_sb[32 * j:32 * (j + 1), m, qsl],
                            start=True, stop=True,
                            tile_position=(32 * j, 0))
                    pt = ppool.tile([128, 2, 512], BF16, tag="pt")
                    nc.scalar.activation(out=pt[:], in_=ps_s[:], func=AF.Exp)
                    em_b = em_sb[:, kc, qsl].unsqueeze(1).broadcast_to(
                        (128, 2, 512))
                    nc.vector.tensor_tensor(out=pt[:], in0=pt[:], in1=em_b,
                                            op=ALU.mult)
                    mm_ctr[0] += 1
                    for hh in range(2):
                        h = 2 * t + hh
                        nc.tensor.matmul(
                            po[t][64 * hh:64 * (hh + 1), :],
                            v_aug[(par, kc)][:, h, :],
                            pt[:, hh, :],
                            start=(kc == 0), stop=(kc == 7),
                            tile_position=(0, 64 * hh),
                            skip_group_check=True)
            # drain: stage po to SBUF, 1/rowsum (rows 32/96) as f32r, then
            # two ones-stationary matmuls broadcast it across each head's
            # partitions in PSUM; one aligned multiply normalizes.
            for tp in range(2):
                rb = psS.tile([128, N], F32, tag="s")
                for u in range(2):
                    t = 2 * tp + u
                    usl = slice(u * 512, (u + 1) * 512)
                    posb = rpool.tile([128, 512], F32, tag="posb")
                    nc.vector.tensor_copy(out=posb[:], in_=po[t][:])
                    rinv = rpool.tile([128, 512], BF16, tag="rinv")
                    nc.vector.reciprocal(out=rinv[:], in_=po[t][:])
                    nc.tensor.matmul(
                        rb[0:64, usl], ones_sb[:, 0, :], rinv[:],
                        start=True, stop=True, tile_position=(0, 0),
                        skip_group_check=True)
                    nc.tensor.matmul(
                        rb[64:128, usl], ones_sb[:, 1, :], rinv[:],
                        start=True, stop=True, tile_position=(0, 64),
                        skip_group_check=True)
                    nc.vector.tensor_tensor(
                        out=otn_sb[:, t, qsl], in0=posb[:], in1=rb[:, usl],
                        op=ALU.mult)
            # output projection for this q-half
            ps_y = psS.tile([128, N], F32, tag="s")
            for qt in range(4):
                for t in range(4):
                    nc.tensor.matmul(
                        ps_y[:, qt * 256:(qt + 1) * 256],
                        otn_sb[0:96, t, qc * 512 + qt * 128: qc * 512 + (qt + 1) * 128],
                        wp_sb[0:96, t, :],
                        start=(t == 0), stop=(t == 3))
            for qt in range(4):
                y_sb = ypool.tile([128, C], F32, tag="y")
                nc.vector.tensor_tensor(
                    out=y_sb[:], in0=ps_y[:, qt * 256:(qt + 1) * 256],
                    in1=bp_sb[:], op=ALU.add)
                nc.sync.dma_start(
                    out=d_y[b, qc * 512 + qt * 128: qc * 512 + (qt + 1) * 128, :],
                    in_=y_sb[:])


def _host_prep(x, mask, affine_matrix, Wq, bq, Wkv, bkv, Wp, bp,
               angle_table, H, W):
    B_, N_, C_ = x.shape
    heads = angle_table.shape[1]
    hd = C_ // heads
    scale = np.float64(hd) ** -0.5
    H = int(H); W = int(W)

    gy, gx = np.meshgrid(np.arange(H, dtype=np.float32),
                         np.arange(W, dtype=np.float32), indexing="ij")
    coords = np.stack([gx.reshape(-1), gy.reshape(-1)], -1).astype(np.float32)
    center = np.array([W / 2.0, H / 2.0], np.float32)
    ego = np.einsum("bij,j->bi", affine_matrix[:, :2, :2], center) \
        + affine_matrix[:, :2, 2]
    rel = coords[None, :, :] - ego[:, None, :]
    ang = np.arctan2(rel[..., 1], rel[..., 0]).astype(np.float32)
    bins = (((ang + np.float32(math.pi)) / np.float32(2.0 * math.pi))
            * (angle_table.shape[0] - 1)).astype(np.int32)
    sig = (1.0 / (1.0 + np.exp(-angle_table[bins]))).astype(np.float32)
    bias = (1.0 + sig).astype(np.float32)                      # (B,N,h)

    bias_const = float(bias.flat[0])
    is_const = bool(np.ptp(bias) < 1e-6 * abs(bias_const))

    import ml_dtypes
    xT = np.ascontiguousarray(x.transpose(0, 2, 1)).astype(np.float16)
    emT = np.ascontiguousarray(
        np.exp(mask).transpose(0, 2, 1).astype(ml_dtypes.bfloat16))  # [k,q]

    # const path: scale*bias folded into Wq/bq; fallback: biasT carries
    # scale*bias and multiplies (x@Wq + bq) on-device.
    qscale = np.float32(scale * bias_const) if is_const else np.float32(1.0)
    Wq_eff = (Wq.astype(np.float64) * qscale).astype(np.float16)
    Wk = np.ascontiguousarray(Wkv[:, :C_]).astype(np.float16)
    Wv = np.ascontiguousarray(Wkv[:, C_:]).astype(np.float16)

    # wp permuted: tile t holds heads (2t, 2t+1) at partitions 0-31 / 64-95
    import ml_dtypes as _mld
    wp_perm = np.zeros((4, 128, C_), _mld.bfloat16)
    for t in range(4):
        wp_perm[t, 0:32] = Wp[32 * (2 * t): 32 * (2 * t) + 32, :]
        wp_perm[t, 64:96] = Wp[32 * (2 * t + 1): 32 * (2 * t + 1) + 32, :]

    bq2 = np.ascontiguousarray((bq * qscale).reshape(2, 128).T, np.float32)
    bk2 = np.ascontiguousarray(bkv[:C_].reshape(2, 128).T, np.float32)
    bv_rep = np.ascontiguousarray(
        np.broadcast_to(bkv[C_:], (128, C_)), np.float32)
    bp_rep = np.ascontiguousarray(
        np.broadcast_to(bp, (128, C_)), np.float32)

    biasT = None
    if not is_const:
        biasT = np.ascontiguousarray(
            np.repeat(bias.transpose(0, 2, 1) * np.float32(scale), hd, axis=1),
            dtype=np.float32)                                   # (B,C,N)
    return xT, emT, Wq_eff, Wk, Wv, wp_perm, bq2, bk2, bv_rep, bp_rep, \
        biasT, is_const


def _ensure_ntff_hook():
    import types
    try:
        from antenv import axon_hooks  # noqa: F401
        return
    except ImportError:
        pass
    import antenv
    mod = types.ModuleType("antenv.axon_hooks")
    _h = {"hook": None}
    mod.get_axon_ntff_profile_hook = lambda: _h["hook"]
    mod.set_axon_ntff_profile_hook = lambda hook: _h.__setitem__("hook", hook)
    sys.modules["antenv.axon_hooks"] = mod
    antenv.axon_hooks = mod
    try:
        sys.path.insert(0, "/root/.axon_site/trn_agent_boot")
        import trn_boot
        hook = trn_boot._ntff_profile_via_ctypes("/opt/axon/libaxon_pjrt.so")
        if hook is not None:
            mod.set_axon_ntff_profile_hook(hook)
    except Exception as e:
        print("ntff hook setup failed:", repr(e))


def _sel_const():
    import ml_dtypes
    sel = np.zeros((128, 2, 64), np.float32)
    sel[32:64, 0, :] = 1.0 / 32.0
    sel[96:128, 1, :] = 1.0 / 32.0
    return sel.astype(ml_dtypes.bfloat16)


def kernel(x, mask, affine_matrix, Wq, bq, Wkv, bkv, Wp, bp,
           angle_table, H, W, _profile=False):
    if _profile:
        _ensure_ntff_hook()
    x = np.asarray(x, np.float32)
    mask = np.asarray(mask, np.float32)
    affine_matrix = np.asarray(affine_matrix, np.float32)
    Wq = np.asarray(Wq, np.float32); bq = np.asarray(bq, np.float32)
    Wkv = np.asarray(Wkv, np.float32); bkv = np.asarray(bkv, np.float32)
    Wp = np.asarray(Wp, np.float32); bp = np.asarray(bp, np.float32)
    angle_table = np.asarray(angle_table, np.float32)

    (xT, emT, Wq_eff, Wk, Wv, wp_perm, bq2, bk2, bv_rep, bp_rep,
     biasT, is_const) = _host_prep(
        x, mask, affine_matrix, Wq, bq, Wkv, bkv, Wp, bp, angle_table, H, W)

    key = "nc_const" if is_const else "nc_bias"
    if key not in _CACHE:
        _CACHE[key] = build_kernel(with_bias=not is_const)
    nc = _CACHE[key]

    in_maps = []
    for m in range(N_CORES):
        bs = [BPC * m + j for j in range(BPC)]
        im = {
            "xT": np.ascontiguousarray(xT[bs]),
            "emT": np.ascontiguousarray(emT[[bb % NW for bb in bs]]),
            "wq": Wq_eff, "wk": Wk, "wv": Wv, "wp": wp_perm,
            "bq": bq2, "bk": bk2, "bv": bv_rep, "bp": bp_rep,
            "ones64": _sel_const(),
        }
        if not is_const:
            im["biasT"] = np.ascontiguousarray(biasT[bs])
        in_maps.append(im)

    res = run_bass_kernel_spmd(nc, in_maps, core_ids=list(range(N_CORES)),
                               trace=_profile)
    out = np.empty((B, N, C), np.float32)
    for m in range(N_CORES):
        y = res.results[m]["y"]
        for j in range(BPC):
            out[BPC * m + j] = y[j]
    if _profile:
        return out, res
    return out


if __name__ == "__main__":
    import reference
    inputs = reference.setup_inputs()
    out = kernel(**{k: (np.asarray(v) if hasattr(v, "shape") else v)
                    for k, v in inputs.items()})
    ref = np.asarray(reference.reference(**inputs))
    err = np.abs(out - ref)
    print("max abs err:", err.max(),
          "absmax-rel:", err.max() / np.abs(ref).max())
